# revision 10
# baseline (speedup 1.0000x reference)
"""NT-Xent loss on 8 Trainium2 cores (v3: fp8 DoubleRow + on-device diag kill).

Math: with row-normalized views zjn, zin and r = [zjn; zin],
S = r@r.T / T, pos_i = (zjn_i . zin_i)/T, the kept logits for row i are
the same-view off-diagonal entries plus pos_i.  All cosine logits are
<= 1/T = 10, so with the fixed shift 10:

  lse_i  = 10 + ln( rowsum_i + epos_i )
  loss   = mean(lse_i - pos_i)

where rowsum_i = sum_{j != i} exp(S_same[i,j] - 10) and
epos_i = exp(pos_i - 10).

Device (SPMD, cores 0-3 view zj, cores 4-7 view zi; each owns a
1024-row slab): rows prescaled by 16, quantized to fp8e4m3.  Per-core
anT columns are rotated by -slab*1024 so each core's own rows occupy
columns [0,1024) -- its Gram diagonal then sits at fixed positions
(tile t, cols t*128..t*128+128, entry [p, t*128+p]) identical across
cores.  An extra identity matmul adds -448*I there before the exp, so
exp(<= -16) ~ 0 removes the diagonal on device (no host cancellation).
G = qnT.T @ anT via DoubleRow matmuls, ACT exp(G*(10/256) - 10) -> bf16,
DVE all-bf16 reduce -> [128,2] partial row sums.  Host does the O(N*D)
rest (normalize, pos, log, mean).
"""

import numpy as np
import ml_dtypes

N = 4096
D = 256
TEMP = 0.1
NCORES = 8
RPC = 2 * N // NCORES          # 1024 rows per core
IT = RPC // 128                # 8 i-tiles of 128 rows
HALF = 2048                    # j-chunk per PSUM buffer / ACT op
NH = N // HALF                 # 2 halves of the 4096-wide Gram row
NCH = HALF // 512              # 4 column chunks per half
SC = 16.0                      # fp8 prescale (power of 2, exact)
ASCALE = (1.0 / TEMP) / (SC * SC)   # 10/256 applied in ACT
DIAGK = 240.0                  # fp8e4m3 max; with idt=2*I the diag gets -480

_CACHE = {}


def _build_program():
    if "nc" in _CACHE:
        return _CACHE["nc"]

    import concourse.bass as bass
    import concourse.tile as tile
    from concourse import bacc, mybir

    F8 = mybir.dt.float8e4
    BF16 = mybir.dt.bfloat16
    F32 = mybir.dt.float32

    nc = bacc.Bacc(
        "TRN2", target_bir_lowering=False, debug=False, num_devices=NCORES
    )

    # anT[h][c][p][k][col] = a8rot[h*2048 + c*512 + col, k*128 + p]
    anT_d = nc.dram_tensor("anT", [NH, NCH, 128, 2, 512], F8, kind="ExternalInput")
    # qnT[p][k][r] = q8slab[r, k*128 + p]
    qnT_d = nc.dram_tensor("qnT", [128, 2, RPC], F8, kind="ExternalInput")
    idt_d = nc.dram_tensor("idt", [128, 128], F8, kind="ExternalInput")
    ngid_d = nc.dram_tensor("ngid", [128, 128], F8, kind="ExternalInput")
    acc_d = nc.dram_tensor("acc", [128, IT, NH], BF16, kind="ExternalOutput")

    with tile.TileContext(nc) as tc:
        with (
            tc.tile_pool(name="weights", bufs=1) as wpool,
            tc.tile_pool(name="scratch", bufs=2) as spool,
            tc.tile_pool(name="psum", bufs=2, space="PSUM") as ppool,
        ):
            qnT = wpool.tile([128, 2, RPC], F8)
            an = [
                [wpool.tile([128, 2, 512], F8, name=f"an{h}_{c}") for c in range(NCH)]
                for h in range(NH)
            ]
            idt = wpool.tile([128, 128], F8)
            ngid = wpool.tile([128, 128], F8)
            # h=0 chunks + qnT on the sync queue (needed first), rest on gpsimd
            nc.sync.dma_start(out=an[0][0][:], in_=anT_d[0, 0])
            nc.sync.dma_start(out=qnT[:], in_=qnT_d[:])
            for c in range(1, NCH):
                nc.sync.dma_start(out=an[0][c][:], in_=anT_d[0, c])
            nc.gpsimd.dma_start(out=idt[:], in_=idt_d[:])
            nc.gpsimd.dma_start(out=ngid[:], in_=ngid_d[:])
            for c in range(NCH):
                nc.gpsimd.dma_start(out=an[1][c][:], in_=anT_d[1, c])

            acc = wpool.tile([128, IT, NH], BF16)
            bias = wpool.tile([128, 1], F32)
            nc.vector.memset(bias[:], -1.0 / TEMP)

            for t in range(IT):
                sc = spool.tile([128, NH, HALF], BF16)
                for h in range(NH):
                    ps = ppool.tile([128, HALF], F32)
                    cd = t // 4 if h == 0 else -1
                    for c in range(NCH):
                        nc.tensor.matmul(
                            ps[:, c * 512:(c + 1) * 512],
                            qnT[:, :, t * 128:(t + 1) * 128],
                            an[h][c][:],
                            start=True,
                            stop=(c != cd),
                            perf_mode=mybir.MatmulPerfMode.DoubleRow,
                            skip_group_check=(c == cd),
                        )
                    if h == 0:
                        nc.tensor.matmul(
                            ps[:, t * 128:(t + 1) * 128],
                            idt[:],
                            ngid[:],
                            start=False,
                            stop=True,
                            skip_group_check=True,
                        )
                    nc.scalar.activation(
                        sc[:, h, :],
                        ps[:],
                        mybir.ActivationFunctionType.Exp,
                        bias=bias[:],
                        scale=ASCALE,
                    )
                with nc.allow_low_precision(
                    reason="bf16 partial row sums; 0.4% rel tolerance is fine"
                ):
                    nc.vector.tensor_reduce(
                        acc[:, t, :],
                        sc[:],
                        axis=mybir.AxisListType.X,
                        op=mybir.AluOpType.add,
                    )

            nc.sync.dma_start(out=acc_d[:], in_=acc[:])

    nc.compile()
    _CACHE["nc"] = nc
    return nc


def _prep_inputs(z_i, z_j):
    f8 = ml_dtypes.float8_e4m3
    zin = z_i / np.sqrt(np.sum(z_i * z_i, axis=1, keepdims=True))
    zjn = z_j / np.sqrt(np.sum(z_j * z_j, axis=1, keepdims=True))
    posn = np.sum(zin * zjn, axis=1, dtype=np.float64) / TEMP      # [4096]

    q8 = [(SC * zjn).astype(f8), (SC * zin).astype(f8)]
    ident = (2.0 * np.eye(128)).astype(f8)
    negid = (-DIAGK * np.eye(128)).astype(f8)

    in_maps = []
    for c in range(NCORES):
        v = c // (NCORES // 2)
        s = c % (NCORES // 2)
        b = q8[v]
        brot = np.roll(b, -s * RPC, axis=0)            # own slab -> cols [0,1024)
        bT = brot.T                                    # [256, 4096]
        anT = np.ascontiguousarray(
            bT.reshape(2, 128, NH, NCH, 512).transpose(2, 3, 1, 0, 4)
        )
        slab = b[s * RPC:(s + 1) * RPC]
        qnT = np.ascontiguousarray(slab.T.reshape(2, 128, RPC).transpose(1, 0, 2))
        in_maps.append({"anT": anT, "qnT": qnT, "idt": ident, "ngid": negid})
    return in_maps, posn


def kernel(z_i, z_j):
    z_i = np.asarray(z_i, dtype=np.float32)
    z_j = np.asarray(z_j, dtype=np.float32)

    from concourse.bass_utils import run_bass_kernel_spmd

    nc = _build_program()
    in_maps, posn = _prep_inputs(z_i, z_j)

    res = run_bass_kernel_spmd(nc, in_maps, list(range(NCORES)))
    _CACHE["last_results"] = res

    rowsum = np.empty(2 * N, dtype=np.float64)
    for c in range(NCORES):
        a = res.results[c]["acc"].astype(np.float64)   # [128, IT, NH]
        slab = a.sum(axis=2).T.reshape(-1)             # [1024], row t*128+p
        rowsum[c * RPC:(c + 1) * RPC] = slab

    posn_g = np.concatenate([posn, posn])
    epos_g = np.exp(posn_g - 1.0 / TEMP)

    lse = 1.0 / TEMP + np.log(rowsum + epos_g)
    loss = np.mean(lse - posn_g)
    return np.array(loss, dtype=np.float32)
